# revision 59
# baseline (speedup 1.0000x reference)
"""Modulated deformable conv (DCNv2-style) Trainium2 Bass kernel.

Batch data-parallel over 8 NeuronCores (1 batch element per core).

Per-core pipeline:
  1. fuse 1x1 conv (PE) -> x kept as CHW padded in SBUF (X2, with a
     col-shifted duplicate on partitions 64-127 for K-stacked conv taps)
     and as TP [x-part, y, 64c] for token stores.
  2. xtok2 HBM token table: row r = (y0+2)*PADW + (x+2) holds
     [pix(y0,x) 64c | pix(y0+1,x) 64c] bf16 (256B). A 512B gather token
     starting at row r covers the 4 bilinear corners (y0,y0+1)x(x0,x0+1)
     because x-adjacent rows are contiguous.
  3. dy/dx/mod convs packed into 54-wide matmuls (2 px-chunks per psum).
  4. map pipeline (DVE/ACT): floor flags, fracs, per-x-corner weight
     tiles WX0/WX1 (rows k: y0-weight, k+9: y1-weight), int16 indices.
  5. dma_gather (transpose=True, elem_step=128, elem_size=256):
     V[128=(2y,64c), xc, npix] per tap.
  6. weights replicated across partitions via PE selector matmuls;
     prd = V * Wrep; contraction on PE with +-2 w_reg folded lhsT.

Column ordering: gather position n <-> map column sigma(n) =
(n%16)*S + n//16; applied at the index wrap DMA, the repl-matmul rhs
view, and the final PSUM->SBUF unpermute copy.
"""
import sys

sys.path.insert(0, "/opt/trn_rl_repo")

from contextlib import ExitStack

import numpy as np

import bass_rust
import concourse.bass as bass
import concourse.bacc as bacc
import concourse.mybir as mybir
from concourse.tile import TileContext
from concourse.mybir import AluOpType as Op
from concourse.mybir import ActivationFunctionType as Act

F32 = mybir.dt.float32
BF16 = mybir.dt.bfloat16
I16 = mybir.dt.int16

W = 128
C = 64
K2 = 9
PADW = 132


def _shape_consts(H):
    NPX = H * W
    NCH = 8
    CH = NPX // NCH          # pixels per chunk
    RPC = CH // W            # image rows per chunk
    S = CH // 16             # wrap cols per chunk slot
    PADH = H + 4
    NTOK = PADH * PADW
    NSPL = max(1, CH // 512)
    SPL = CH // NSPL         # matmul N per split (<=512)
    return NPX, NCH, CH, RPC, S, PADH, NTOK, NSPL, SPL


def build_nc(H=128, num_devices=8, gw=512, wrepp_bufs=4, pat=None,
             VVB=3, PRDB=10, WRB=10, CLAGV=10):
    NPX, NCH, CH, RPC, S, PADH, NTOK, NSPL, SPL = _shape_consts(H)
    RSPL = SPL // W                      # image rows per split
    NTOKP = ((NTOK + PADW + 2047) // 2048) * 2048

    es = ExitStack()
    nc = bacc.Bacc("TRN2", target_bir_lowering=False, debug=False,
                   num_devices=num_devices)

    x_img = nc.dram_tensor("x_img", [C, NPX], BF16, kind="ExternalInput")
    x_cont = nc.dram_tensor("x_cont", [C, NPX], BF16, kind="ExternalInput")
    out = nc.dram_tensor("out", [C, NPX], BF16, kind="ExternalOutput")

    fuse_lhsT = nc.dram_tensor("fuse_lhsT", [128, 64], BF16, kind="ExternalInput")
    pw, sw = {}, {}
    for ty in range(3):
        pw[ty] = nc.dram_tensor(f"pw_{ty}", [128, 96], BF16,
                                kind="ExternalInput")
        sw[ty] = nc.dram_tensor(f"sw_{ty}", [64, 96], BF16,
                                kind="ExternalInput")
    dcols = {}
    for nm in ("bias_dy", "bias_dx", "bias_mod", "xw_s1", "xw_s2"):
        dcols[nm] = nc.dram_tensor(nm, [128, 1], F32, kind="ExternalInput")
    bt0 = nc.dram_tensor("bt0", [128, 2 * CH], F32, kind="ExternalInput")
    sel = nc.dram_tensor("sel", [128, K2 * 128], BF16, kind="ExternalInput")
    regc = nc.dram_tensor("regc", [128, K2 * 64], BF16, kind="ExternalInput")

    xtok2 = nc.dram_tensor("xtok2", [NTOKP, 128], BF16, kind="Internal")
    xtok4 = xtok2.ap()[0:NTOK, :].rearrange("(a b) e -> a b e", b=PADW)
    # overlapping-window view for the gather: idx r -> elems r*128..+256
    gat_ap = bass_rust.AP(xtok2.ap().tensor, 0, [[128, NTOKP - 1], [1, 256]])

    MM = lambda *a, **k: nc.tensor.matmul(*a, **k)

    with TileContext(nc) as tc:
        pconst = es.enter_context(tc.tile_pool(name="pconst", bufs=1))
        pp = es.enter_context(tc.tile_pool(name="pp", bufs=1))
        pv = es.enter_context(tc.tile_pool(name="pv", bufs=1))

        # ---- stage constants
        fuse_w = pconst.tile([128, 64], BF16)
        nc.sync.dma_start(fuse_w[:], fuse_lhsT.ap())
        conv_w = {}
        for ty in range(3):
            tP = pconst.tile([128, 96], BF16, name=f"cwp_{ty}")
            nc.sync.dma_start(tP[:], pw[ty].ap())
            tS = pconst.tile([64, 96], BF16, name=f"cws_{ty}")
            nc.sync.dma_start(tS[:], sw[ty].ap())
            conv_w[ty] = (tP, tS)
        col = {}
        for nm in ("bias_dy", "bias_dx", "bias_mod", "xw_s1", "xw_s2"):
            t = pconst.tile([128, 1], F32, name=f"c_{nm}")
            nc.sync.dma_start(t[:], dcols[nm].ap())
            col[nm] = t
        btok0 = pconst.tile([128, CH], F32)
        nc.sync.dma_start(btok0[:], bt0.ap()[:, 0:CH])
        sel_sb = pconst.tile([128, K2 * 128], BF16)
        nc.sync.dma_start(sel_sb[:], sel.ap())
        regsb = pconst.tile([128, K2 * 64], BF16)
        nc.sync.dma_start(regsb[:], regc.ap())

        WX, IDXT, DEFER = {}, {}, {}
        WIDX = pp.tile([128, K2 * 8 * S], I16, name="widx")

        with tc.tile_pool(name="pX", bufs=1) as pX:
            X2 = pp.tile([128, PADH, PADW], BF16, name="X2")

            # =============== phase 0 ===============
            with tc.tile_pool(name="pin", bufs=1) as pin, \
                 tc.tile_pool(name="p0ps", bufs=2, space="PSUM") as p0ps:
                instk = pin.tile([128, NPX], BF16)
                for hx in range(2):
                    cs = hx * (NPX // 2)
                    nc.sync.dma_start(instk[0:64, cs:cs + NPX // 2],
                                      x_img.ap()[:, cs:cs + NPX // 2])
                    nc.sync.dma_start(instk[64:128, cs:cs + NPX // 2],
                                      x_cont.ap()[:, cs:cs + NPX // 2])

                # X2 pad ring (interior fully overwritten below)
                nc.vector.memset(X2[:, 0:2, :], 0.0)
                nc.vector.memset(X2[:, H + 2:H + 4, :], 0.0)
                nc.vector.memset(X2[0:64, :, 0:2], 0.0)
                nc.vector.memset(X2[64:128, :, 128:132], 0.0)

                TP = pin.tile([128, PADH, 64], BF16, name="TP")
                nc.vector.memset(TP[:, 0:2, :], 0.0)
                nc.vector.memset(TP[:, H + 2:H + 4, :], 0.0)

                # zero pads of xtok2: top band, bottom band, side strips
                zt = pin.tile([128, 256], BF16)
                nc.vector.memset(zt[:, :], 0.0)
                zb = [(0, PADW), (NTOK - 2 * PADW, 2 * PADW + PADW)]
                for r0, n in zb:
                    while n > 0:
                        cr = min(128, n)
                        nc.sync.dma_start(xtok2.ap()[r0:r0 + cr, :],
                                          zt[0:cr, 0:128])
                        r0 += cr
                        n -= cr
                # side strips: cols {0,1} and {130,131} of rows 1..129
                for c0 in (0, PADW - 2):
                    for rr, nr in ((1, 64), (65, 65)):
                        nc.sync.dma_start(
                            xtok4[rr:rr + nr, c0:c0 + 2, :],
                            zt[0:nr, 0:256])

                # transposed fuse -> TP rows (first, so token stores
                # stream out while the X2 fuse runs); 4 rows per psum
                tpap = TP[:, :, :]

                def _tok_store(rb, nr):
                    src = bass_rust.AP(
                        tpap.tensor, tpap.offset + (1 + rb) * 64,
                        [[PADH * 64, 128], [64, nr], [1, 128]])
                    nc.sync.dma_start(
                        xtok4[1 + rb:1 + rb + nr, 2:130, :].transpose([1, 0, 2]),
                        src)

                for i4 in range(H // 4):
                    pst = p0ps.tile([128, 4, 64], F32, tag="fuseT", bufs=4)
                    for r in range(4):
                        i = i4 * 4 + r
                        MM(pst[:, r, :], instk[:, i * W:(i + 1) * W],
                           fuse_w[:, :], start=True, stop=True,
                           skip_group_check=True)
                    if i4 % 2 == 0:
                        nc.vector.tensor_copy(TP[:, 2 + 4 * i4:6 + 4 * i4, :],
                                              pst[:, :, :])
                    else:
                        nc.scalar.copy(TP[:, 2 + 4 * i4:6 + 4 * i4, :],
                                       pst[:, :, :])
                    if i4 == 11:
                        _tok_store(0, 43)
                    elif i4 == 22:
                        _tok_store(43, 43)
                if True:
                    _tok_store(86, 43)

                # fuse conv -> X2 rows 0-63 interior (+ col-shifted dup)
                for c8 in range(NCH):
                    for j in range(NSPL):
                        ps = p0ps.tile([64, SPL], F32, tag="fuseps", bufs=4)
                        off = c8 * CH + j * SPL
                        MM(ps[:], fuse_w[:, :], instk[:, off:off + SPL],
                           start=True, stop=True)
                        i0 = off // W
                        ps3 = ps[:].rearrange("p (a b) -> p a b", b=W)
                        nc.scalar.copy(X2[0:64, 2 + i0:2 + i0 + RSPL, 2:130], ps3)
                        nc.vector.tensor_copy(
                            X2[64:128, 2 + i0:2 + i0 + RSPL, 0:128], ps3)

            # =============== phase A: convs + maps ===============
            with tc.tile_pool(name="paps", bufs=2, space="PSUM") as paps, \
                 tc.tile_pool(name="pam", bufs=1) as pam:
                for g in range(2):
                    # packed conv: dy@0 | dx@32 | mod@64 in 96 lhsT cols
                    # (32-aligned for PSUM reads); one px-chunk per psum
                    qsb = {}
                    for q in ("dy", "dx", "mod"):
                        qsb[q] = pam.tile([128, CH], BF16, tag=f"q_{q}",
                                          name=f"qsb_{q}{g}")
                    for cb in range(4):
                        qps = paps.tile([128, CH], F32, tag="convps")
                        for j in range(NSPL):
                            ist = (g * 4 + cb) * RPC + j * RSPL
                            dst = qps[0:96, j * SPL:(j + 1) * SPL]
                            for ty in range(3):
                                tP, tS = conv_w[ty]
                                MM(dst,
                                   tP[:, :],
                                   X2[0:128, 1 + ist + ty:1 + ist + ty + RSPL,
                                      1:1 + W],
                                   start=(ty == 0), stop=False)
                                MM(dst,
                                   tS[:, :],
                                   X2[0:64, 1 + ist + ty:1 + ist + ty + RSPL,
                                      2:2 + W],
                                   start=False, stop=(ty == 2))
                        # dy bias: DVE for g0, Act for g1 (g1 phase A
                        # runs concurrently with phase C where DVE paces)
                        if g == 0:
                            nc.vector.tensor_scalar(
                                qsb["dy"][32 * cb:32 * cb + 32, :],
                                qps[0:32, :],
                                col["bias_dy"][32 * cb:32 * cb + 32], None,
                                Op.add)
                        else:
                            nc.scalar.activation(
                                qsb["dy"][32 * cb:32 * cb + 32, :],
                                qps[0:32, :], Act.Identity,
                                bias=col["bias_dy"][32 * cb:32 * cb + 32],
                                scale=1.0)
                        for qoff, q in ((32, "dx"), (64, "mod")):
                            bias = col["bias_" + q]
                            nc.scalar.activation(
                                qsb[q][32 * cb:32 * cb + 32, :],
                                qps[qoff:qoff + 32, :],
                                Act.Sigmoid if q == "mod" else Act.Identity,
                                bias=bias[32 * cb:32 * cb + 32],
                                scale=1.0)

                    # index chain first: unblocks WIDX wrap DMAs early
                    FY = pam.tile([128, CH], BF16, tag="m1")
                    nc.vector.tensor_scalar(FY[:], qsb["dy"][:], 0.0, None, Op.is_lt)
                    FX = pam.tile([128, CH], BF16, tag="m2")
                    nc.vector.tensor_scalar(FX[:], qsb["dx"][:], 0.0, None, Op.is_lt)
                    # idx = btok0 - 132*FY - FX
                    T1 = pam.tile([128, CH], F32, tag="m8")
                    nc.vector.scalar_tensor_tensor(
                        T1[:], FY[:], -float(PADW),
                        btok0[:, :], Op.mult, Op.add)
                    if g == 0:
                        # reload the g1 half over the same buffer
                        nc.sync.dma_start(btok0[:], bt0.ap()[:, CH:2 * CH])
                    T2 = pam.tile([128, CH], F32, tag="m9")
                    nc.vector.tensor_tensor(T2[:], T1[:], FX[:], Op.subtract)
                    idx0 = pp.tile([128, CH], I16, name=f"idx0_{g}")
                    nc.vector.tensor_copy(idx0[:], T2[:])
                    IDXT[g] = idx0

                    RY = pam.tile([128, CH], BF16, tag="m3")
                    nc.vector.tensor_tensor(RY[:], qsb["dy"][:], FY[:], Op.add)
                    RX = pam.tile([128, CH], BF16, tag="m4")
                    nc.vector.tensor_tensor(RX[:], qsb["dx"][:], FX[:], Op.add)
                    # YW rows k: 1-ry ; rows k+9: ry   (xw_s1/s2 pattern)
                    YW = pam.tile([128, CH], BF16, tag="m5")
                    nc.vector.tensor_scalar(YW[:], RY[:], col["xw_s1"][:],
                                            col["xw_s2"][:], Op.mult, Op.add)
                    wx0 = pp.tile([128, CH], BF16, name=f"wx0_{g}")
                    wx1 = pp.tile([128, CH], BF16, name=f"wx1_{g}")
                    if g == 0:
                        MRX = pam.tile([128, CH], BF16, tag="m6")
                        nc.vector.tensor_tensor(MRX[:], qsb["mod"][:], RX[:],
                                                Op.mult)
                        MNRX = pam.tile([128, CH], BF16, tag="m7")
                        nc.vector.tensor_tensor(MNRX[:], qsb["mod"][:], MRX[:],
                                                Op.subtract)
                        nc.vector.tensor_tensor(wx0[:], YW[:], MNRX[:], Op.mult)
                        nc.vector.tensor_tensor(wx1[:], YW[:], MRX[:], Op.mult)
                    else:
                        # defer g1's weight tail into the phase-C gap
                        YW1 = pp.tile([128, CH], BF16, name="yw1")
                        nc.vector.tensor_copy(YW1[:], YW[:])
                        MG1 = pp.tile([128, CH], BF16, name="mg1")
                        nc.vector.tensor_copy(MG1[:], qsb["mod"][:])
                        RX1 = pp.tile([128, CH], BF16, name="rx1")
                        nc.vector.tensor_copy(RX1[:], RX[:])
                        DEFER["g1wx"] = (YW1, MG1, RX1, wx0, wx1)
                    WX[(g, 0)], WX[(g, 1)] = wx0, wx1

                    # wrap + replicate this g's indices immediately so the
                    # first gathers can start while g=1 is still computing;
                    # replicate in two col-chunks so k=0..3 gathers go early
                    HW_ = K2 * 4 * S
                    for kc in (range(0, 4), range(4, K2)):
                        for k in kc:
                            for cb in range(4):
                                slot = ((g * K2 + k) * 4 + cb) * S
                                sap = IDXT[g][32 * cb + k:32 * cb + k + 1, :]
                                eng = nc.sync if (k + cb) % 2 == 0 else nc.gpsimd
                                eng.dma_start(
                                    WIDX[0:16, slot:slot + S],
                                    sap.rearrange("p (a b) -> p a b", b=S))
                        c0 = (g * K2 + kc[0]) * 4 * S
                        c1 = (g * K2 + kc[-1] + 1) * 4 * S
                        for r8 in range(1, 8):
                            nc.sync.dma_start(
                                WIDX[16 * r8:16 * r8 + 16, c0:c1],
                                WIDX[0:16, c0:c1])

        # =============== phase C: gather / weight / contract ===============
        GW = gw
        NH = CH // GW
        NSUB = GW // SPL
        if pat is None:
            # per-tile engine pattern: 'a' Act-mat + DVE tt,
            # 'p' Act-mat + Pool tt, 'd' DVE-direct-from-psum tt
            pat = "adpdadpdadpddpdd"
        with tc.tile_pool(name="pcps", bufs=2, space="PSUM") as pcps, \
             tc.tile_pool(name="pops", bufs=1, space="PSUM") as pops, \
             tc.tile_pool(name="pc", bufs=3) as pc:
            for g in range(2):
                for hh in range(2):          # half-group: chunks (2hh, 2hh+1)
                    outp = pops.tile([128, CH], F32, tag="outp", bufs=1)
                    pend, CLAG = [], CLAGV
                    qpat = pat if isinstance(pat, str) else pat[g * 2 + hh]
                    nmat = [0]
                    for k in range(K2):
                        vv = pv.tile([128, 2, 2 * CH], BF16, tag="vt", bufs=VVB,
                                     name="vv")
                        islot = ((g * K2 + k) * 4 + 2 * hh) * S
                        nc.gpsimd.dma_gather(
                            vv[:, :, :], gat_ap,
                            WIDX[:, islot:islot + 2 * S],
                            num_idxs=2 * CH, num_idxs_reg=2 * CH,
                            elem_size=256, elem_step=128, transpose=True,
                            single_packet=False)
                        for xc in (0, 1):
                            for ci in range(2):
                                cb = 2 * hh + ci
                                wx = WX[(g, xc)]
                                cyv = wx[32 * cb:32 * cb + 18, :].rearrange(
                                    "p (a b) -> p b a", b=S)   # [18, S, 16]
                                for h in range(NH):
                                    wrepp = pcps.tile([128, GW], F32,
                                                      tag="wrepp",
                                                      bufs=wrepp_bufs)
                                    for u in range(NSUB):
                                        q0 = (h * GW + u * SPL) // 16
                                        MM(wrepp[:, u * SPL:(u + 1) * SPL],
                                           sel_sb[32 * cb:32 * cb + 18,
                                                  k * 128:(k + 1) * 128],
                                           cyv[:, q0:q0 + SPL // 16, :],
                                           start=True, stop=True,
                                           tile_position=(32 * cb, 0),
                                           skip_group_check=True)
                                    prd = pc.tile([128, GW], BF16, tag="prd",
                                                  bufs=PRDB)
                                    vsl = vv[:, xc,
                                             ci * CH + h * GW:
                                             ci * CH + (h + 1) * GW]
                                    r = qpat[nmat[0] % len(qpat)]
                                    nmat[0] += 1
                                    if r in "ap":   # Act materialize
                                        wreps = pc.tile([128, GW], BF16,
                                                        tag="wreps", bufs=WRB)
                                        nc.scalar.copy(wreps[:], wrepp[:])
                                        teng = (nc.vector if r == "a"
                                                else nc.gpsimd)
                                        teng.tensor_tensor(
                                            prd[:], vsl, wreps[:], Op.mult)
                                    else:           # DVE direct from psum
                                        nc.vector.tensor_tensor(
                                            prd[:], vsl, wrepp[:], Op.mult)

                                    def _emit_contr(prd=prd, xc=xc, k=k, ci=ci,
                                                    h=h, outp=outp, GW=GW,
                                                    NSUB=NSUB):
                                        for u in range(NSUB):
                                            MM(outp[64 * ci:64 * ci + 64,
                                                    h * GW + u * SPL:
                                                    h * GW + (u + 1) * SPL],
                                               regsb[:, k * 64:(k + 1) * 64],
                                               prd[:, u * SPL:(u + 1) * SPL],
                                               start=(k == 0 and xc == 0),
                                               stop=(k == K2 - 1 and xc == 1),
                                               skip_group_check=True)
                                    pend.append(_emit_contr)
                                    if len(pend) > CLAG:
                                        pend.pop(0)()
                    for fe in pend:
                        fe()
                    if g == 0 and hh == 0 and "g1wx" in DEFER:
                        YW1, MG1, RX1, wx0d, wx1d = DEFER.pop("g1wx")
                        MRX1 = pc.tile([128, CH], BF16, tag="mrx1", bufs=1)
                        nc.vector.tensor_tensor(MRX1[:], MG1[:], RX1[:],
                                                Op.mult)
                        MNRX1 = pc.tile([128, CH], BF16, tag="mnrx1", bufs=1)
                        nc.vector.tensor_tensor(MNRX1[:], MG1[:], MRX1[:],
                                                Op.subtract)
                        nc.vector.tensor_tensor(wx0d[:], YW1[:], MNRX1[:],
                                                Op.mult)
                        nc.vector.tensor_tensor(wx1d[:], YW1[:], MRX1[:],
                                                Op.mult)
                    for ci in range(2):
                        cb = 2 * hh + ci
                        c8 = g * 4 + cb
                        outs = pc.tile([64, CH], BF16, tag="outs", bufs=2)
                        # out col m = p*S + q <- outp col n = q*16 + p
                        opv = outp[64 * ci:64 * ci + 64, :].rearrange(
                            "p (q a) -> p a q", a=16)       # [64, 16, S]
                        nc.scalar.copy(
                            outs[:].rearrange("p (a q) -> p a q", a=16), opv)
                        nc.sync.dma_start(out.ap()[:, c8 * CH:(c8 + 1) * CH],
                                          outs[:])
        es.close()

    nc.compile()
    return nc


# ======================= host-side preparation =======================

def _host_consts(w_fuse, w_off, b_off, w_mod, b_mod, w_reg, H=128):
    NPX, NCH, CH, RPC, S, PADH, NTOK, NSPL, SPL = _shape_consts(H)
    import ml_dtypes
    bf = lambda x: np.asarray(x, np.float32).astype(ml_dtypes.bfloat16)

    consts = {}
    wf = np.asarray(w_fuse, np.float32).reshape(64, 128)
    consts["fuse_lhsT"] = bf(np.ascontiguousarray(wf.T))

    w_off = np.asarray(w_off, np.float32).reshape(18, 64, 3, 3)
    w_mod = np.asarray(w_mod, np.float32).reshape(9, 64, 3, 3)

    def qw(q, k):
        return (w_off[2 * k] if q == "dy"
                else w_off[2 * k + 1] if q == "dx" else w_mod[k])

    for ty in range(3):
        P = np.zeros((128, 96), np.float32)
        Sg = np.zeros((64, 96), np.float32)
        for qi, q in enumerate(("dy", "dx", "mod")):
            for m in range(18):
                k = m % 9
                P[0:64, 32 * qi + m] = qw(q, k)[:, ty, 0]
                P[64:128, 32 * qi + m] = qw(q, k)[:, ty, 2]
                Sg[0:64, 32 * qi + m] = qw(q, k)[:, ty, 1]
        consts[f"pw_{ty}"] = bf(P)
        consts[f"sw_{ty}"] = bf(Sg)

    b_off = np.asarray(b_off, np.float32)
    b_mod = np.asarray(b_mod, np.float32)
    bdy = np.zeros((128, 1), np.float32)
    bdx = np.zeros((128, 1), np.float32)
    bmd = np.zeros((128, 1), np.float32)
    s1 = np.zeros((128, 1), np.float32)
    s2 = np.zeros((128, 1), np.float32)
    for r in range(128):
        rr = r % 32
        if rr < 18:
            k = rr % 9
            bdy[r] = b_off[2 * k]
            bdx[r] = b_off[2 * k + 1]
            bmd[r] = b_mod[k]
        if rr < 9:
            s1[r], s2[r] = -1.0, 1.0
        elif rr < 18:
            s1[r], s2[r] = 1.0, 0.0
    consts["bias_dy"], consts["bias_dx"], consts["bias_mod"] = bdy, bdx, bmd
    consts["xw_s1"], consts["xw_s2"] = s1, s2

    # token index base: (y+ky+1)*PADW + (x+kx+1)
    b0 = np.zeros((128, 2 * CH), np.float32)
    for r in range(128):
        cb = r // 32
        rr = r % 32
        k = rr % 9 if rr < 18 else 0
        ky, kx = k // 3, k % 3
        for g in range(2):
            c8 = g * 4 + cb
            cols = np.arange(CH)
            px = c8 * CH + cols
            i, j = px // W, px % W
            b0[r, g * CH:(g + 1) * CH] = (i + ky + 1) * PADW + (j + kx + 1)
    consts["bt0"] = b0

    selm = np.zeros((128, K2 * 128), np.float32)
    for cb in range(4):
        for k in range(K2):
            selm[32 * cb + k, k * 128:k * 128 + 64] = 1.0
            selm[32 * cb + k + 9, k * 128 + 64:k * 128 + 128] = 1.0
    consts["sel"] = bf(selm)

    w_reg = np.asarray(w_reg, np.float32).reshape(64, 64, 3, 3)
    rg = np.zeros((128, K2 * 64), np.float32)
    for k in range(K2):
        ky, kx = k // 3, k % 3
        blkT = w_reg[:, :, ky, kx].T       # [c, o]
        rg[0:64, k * 64:(k + 1) * 64] = 2.0 * blkT
        rg[64:128, k * 64:(k + 1) * 64] = 2.0 * blkT
    consts["regc"] = bf(rg)
    return consts


_NC_CACHE = {}


def kernel(x_img, x_cont, w_fuse, w_off, b_off, w_mod, b_mod, w_reg):
    from concourse.bass_utils import run_bass_kernel_spmd
    import ml_dtypes

    H = 128
    B = int(x_img.shape[0])
    NPX = H * W
    if "nc" not in _NC_CACHE:
        _NC_CACHE["nc"] = build_nc(H=H, num_devices=8)
    nc = _NC_CACHE["nc"]

    consts = _host_consts(w_fuse, w_off, b_off, w_mod, b_mod, w_reg, H=H)
    x_img = np.asarray(x_img, np.float32).astype(ml_dtypes.bfloat16)
    x_cont = np.asarray(x_cont, np.float32).astype(ml_dtypes.bfloat16)
    in_maps = []
    for b in range(B):
        m = dict(consts)
        m["x_img"] = np.ascontiguousarray(x_img[b].reshape(C, NPX))
        m["x_cont"] = np.ascontiguousarray(x_cont[b].reshape(C, NPX))
        in_maps.append(m)

    res = run_bass_kernel_spmd(nc, in_maps, core_ids=list(range(B)))
    outs = [np.asarray(res.results[b]["out"]).astype(np.float32).reshape(C, H, W)
            for b in range(B)]
    return np.stack(outs)
